# revision 18
# baseline (speedup 1.0000x reference)
"""Trainium2 Bass kernel for nn_MemoryNetwork (scatter_memory).

Computation (reference, per batch row b):
    f = feature / ||feature||                       [B, 768]
    topic = f @ W_topic.T ; dom = f @ W_domain.T    [B, 256]
    att   = softmax_m(TAU * topic . memory[d,m])    [B, 9, 10]
    sep   = sum_m att * memory[d,m]                 [B, 9, 256]
    out   = softmax_d(TAU * sep . dom)              [B, 1, 9]

Reformulation: fold the tiny memory banks and TAU into the projections on
the host (f normalized host-side too, so no per-row scale on device):
    RS = TAU * mem_flat @ W_topic   [90, 768]
    RT = TAU * mem_flat @ W_domain  [90, 768]
    rawS = fn @ RS.T ; rawT = fn @ RT.T             [B, 90] each
    ex   = exp(rawS - SHIFT)          (softmax_m numerator; logits in
                                       [-123, 105] so a const shift is safe)
    datt = (sum_m ex*rawT) / (sum_m ex)
    out  = softmax_d(datt)            (const shift again)

Precision: rawS feeds an exponent with TAU-amplified spread, so it needs
~2^-15 relative accuracy on f; rawT only enters linearly and tolerates
plain fp16. Scheme (validated host-side, rel err 7.0e-3 vs 2e-2 gate):
    fhi  = fp16(fn);  flo8 = e4m3((fn - fhi) * 2^17)   (DMA: 3 B/elem)
    RhiS/RloS = fp16 split of RS;  RhiT = fp16(RT);  RS8 = e4m3(RS)
    rawS = fhi@RhiS + fhi@RloS + (flo8@RS8) * 2^-17;  rawT = fhi@RhiT
Per 128-contraction chunk that is 3 PE matmuls: 180-col fp16 [RhiS|RhiT],
90-col fp16 RloS (PSUM-accumulated onto the S columns), 90-col fp8 into a
separate bank (carries the 2^17 scale). LDWEIGHTS fully hides under the
streams (measured), so the PE floor is ~360 cols/chunk = 150 ns warm.

Sharding: data-parallel over B across 8 cores (4096 rows each). All DRAM
layouts are pre-tiled host-side so every DMA descriptor is >=3 KB
contiguous per partition. A burst of dependency-free warmup matmuls at
t=0 starts the PE HAM clock ramp (1.2 -> 2.4 GHz) while DMA fills.
"""

import sys

sys.path.insert(0, "/opt/trn_rl_repo")

import numpy as np

B, IN, E, D, M = 32768, 768, 256, 9, 10
NCORES = 8
BC = B // NCORES  # rows per core
P = 128           # partition tile
NT = BC // P      # batch tiles per core (32)
G = 8             # tiles per softmax group
NG = NT // G      # groups (4)
HB = 4            # tiles per DMA half-block
KC = IN // P      # contraction chunks (6)
DM = D * M        # 90
TAU = 32.0
SHIFT = 50.0
S8 = 2.0 ** 17    # flo8 pre-scale
N_WARM = 50       # HAM warmup matmuls (bridge the DMA fill, trip the ramp)

_CACHE: dict = {}


def _build_nc(repeat=1):
    from contextlib import ExitStack

    import concourse.bacc as bacc
    import concourse.tile as tile
    from concourse import mybir

    F32 = mybir.dt.float32
    F16 = mybir.dt.float16
    F8 = mybir.dt.float8e4
    AF = mybir.ActivationFunctionType
    ALU = mybir.AluOpType

    nc = bacc.Bacc(trn_type="TRN2")
    fhi = nc.dram_tensor("fhi", [P, NT, KC, P], F16, kind="ExternalInput")
    flo8 = nc.dram_tensor("flo8", [P, NT, KC, P], F8, kind="ExternalInput")
    # rt columns: 0:90 RhiS, 90:180 RhiT, 180:270 RloS
    rt = nc.dram_tensor("rt", [P, KC, 3 * DM], F16, kind="ExternalInput")
    rt8 = nc.dram_tensor("rt8", [P, KC, DM], F8, kind="ExternalInput")
    out = nc.dram_tensor("out", [P, NT, D], F32, kind="ExternalOutput")

    with tile.TileContext(nc) as tc, ExitStack() as ctx:
        const = ctx.enter_context(tc.tile_pool(name="const", bufs=1))
        # All feature planes fit SBUF (9.4 MB) — buffer everything and let
        # the DMA rings run gap-free instead of self-pacing behind the PE.
        fpool = ctx.enter_context(tc.tile_pool(name="fts", bufs=1))
        lpool = ctx.enter_context(tc.tile_pool(name="lts", bufs=1))
        stg = ctx.enter_context(tc.tile_pool(name="stg", bufs=2))
        gpool = ctx.enter_context(tc.tile_pool(name="grp", bufs=2))
        spool = ctx.enter_context(tc.tile_pool(name="small", bufs=2))
        apool = ctx.enter_context(tc.tile_pool(name="aps", bufs=4, space="PSUM"))
        bpool = ctx.enter_context(tc.tile_pool(name="bps", bufs=2, space="PSUM"))
        wpool = ctx.enter_context(tc.tile_pool(name="wps", bufs=1, space="PSUM"))

        rt_sb = const.tile([P, KC, 3 * DM], F16)
        rt8_sb = const.tile([P, KC, DM], F8)
        bias_shift = const.tile([P, 1], F32)
        out_sb = const.tile([P, NT, D], F32)
        wz = const.tile([P, P], F16)

        # Feature tiles: everything resident. Group 0 split per half-block
        # (and per tile on the wire) so the first matmuls start early;
        # groups 1-3 are single full-group transfers.
        f0 = fpool.tile([P, HB, KC, P], F16, tag="f0")
        f1 = fpool.tile([P, HB, KC, P], F16, tag="f1")
        l0 = lpool.tile([P, HB, KC, P], F8, tag="l0")
        l1 = lpool.tile([P, HB, KC, P], F8, tag="l1")
        fgs = [
            fpool.tile([P, G, KC, P], F16, tag=f"fg{i}", name=f"fg{i}")
            for i in (1, 2, 3)
        ]
        lgs = [
            lpool.tile([P, G, KC, P], F8, tag=f"lg{i}", name=f"lg{i}")
            for i in (1, 2, 3)
        ]

        def ftile(t):
            if t < HB:
                return f0, t
            if t < G:
                return f1, t - HB
            return fgs[t // G - 1], t % G

        def ltile(t):
            if t < HB:
                return l0, t
            if t < G:
                return l1, t - HB
            return lgs[t // G - 1], t % G

        # Memsets on the vector engine: warmup needs wz immediately and the
        # gpsimd queue must stay pure DMA.
        nc.vector.memset(wz[:], 0.0)
        nc.vector.memset(bias_shift[:], -SHIFT)

        # The kernel is DMA-throughput-bound (~270 GB/s effective/core vs
        # a 240 GB/s steady PE demand), so the stream must be dense and
        # need-ordered from the start. Issue everything up front in need
        # order, greedily balancing bytes across the two rings (sync HWDGE
        # + gpsimd SWDGE; nc.scalar would steal ACT time).
        FB = KC * P * 2          # fp16 bytes per feature tile per partition
        xfers = []               # (bytes, dst_ap, src_ap) in need order
        xfers.append((P * 2 * 270 * 2, rt_sb[:, 0:2, :], rt[:, 0:2, :]))
        xfers.append((P * KC * DM, rt8_sb[:], rt8[:, :, :]))
        for t in range(HB):
            xfers.append((P * FB, f0[:, t : t + 1], fhi[:, t : t + 1]))
            xfers.append((P * FB // 2, l0[:, t : t + 1], flo8[:, t : t + 1]))
            if t < 2:
                sl = slice(2 + 2 * t, 4 + 2 * t)
                xfers.append((P * 2 * 270 * 2, rt_sb[:, sl, :], rt[:, sl, :]))
        xfers.append((P * HB * FB, f1[:], fhi[:, HB : 2 * HB]))
        xfers.append((P * HB * FB // 2, l1[:], flo8[:, HB : 2 * HB]))
        for i in (1, 2, 3):
            xfers.append((P * G * FB, fgs[i - 1][:], fhi[:, i * G : (i + 1) * G]))
            xfers.append((P * G * FB // 2, lgs[i - 1][:],
                          flo8[:, i * G : (i + 1) * G]))

        cum = [0, 0]
        engs = [nc.sync, nc.gpsimd]
        for nbytes, dst, src in xfers:
            ring = 0 if cum[0] <= cum[1] else 1
            engs[ring].dma_start(dst, src)
            cum[ring] += nbytes

        # HAM warmup: dependency-free matmuls keep the PE busy from t~0 so
        # the 2.4 GHz un-throttle fires while the first feature DMA lands.
        wps = wpool.tile([P, DM], F32)
        for _ in range(N_WARM):
            nc.tensor.matmul(wps[:], wz[:], wz[:, 0:DM], start=True, stop=True)

        for g in range(NG * repeat):
            g = g % NG
            ex_g = gpool.tile([P, G, DM], F32, tag="exg")
            # staged [rawS_main | rawT] per tile, copied from PSUM by ACT
            as_g = gpool.tile([P, G, 2 * DM], F32, tag="asg")
            sums = spool.tile([P, G, D], F32, tag="sums")
            wsum = spool.tile([P, G, D], F32, tag="wsum")

            for h in range(G // HB):
                hb = g * (G // HB) + h
                rstg = stg.tile([P, HB, DM], F32, tag="rstg")
                # fp8 corrections for all HB tiles share one PSUM bank
                Bp = bpool.tile([P, HB, DM], F32, tag="B")

                for s in range(HB):
                    sg = h * HB + s  # tile index within group
                    t = hb * HB + s
                    fh, fi = ftile(t)
                    lh, li = ltile(t)
                    A = apool.tile([P, 2 * DM], F32, tag="A")
                    for k in range(KC):
                        # A[:, 0:90] += fhi@RhiS ; A[:, 90:180] += fhi@RhiT
                        nc.tensor.matmul(
                            A[:], fh[:, fi, k, :], rt_sb[:, k, 0 : 2 * DM],
                            start=(k == 0), stop=False,
                        )
                        # A[:, 0:90] += fhi@RloS  (same-column accumulate)
                        nc.tensor.matmul(
                            A[:, 0:DM], fh[:, fi, k, :],
                            rt_sb[:, k, 2 * DM : 3 * DM],
                            start=False, stop=(k == KC - 1),
                            skip_group_check=True,
                        )
                        # Bp[:, s] += flo8@RS8   (scaled by 2^17)
                        nc.tensor.matmul(
                            Bp[:, s, :], lh[:, li, k, :], rt8_sb[:, k, :],
                            start=(k == 0), stop=(k == KC - 1),
                        )
                    # stage [rawS_main | rawT] to SBUF (frees the PSUM bank)
                    nc.scalar.copy(as_g[:, sg, :], A[:])

                # one fused rawS = Bp * 2^-17 + A_S per half-block
                nc.vector.scalar_tensor_tensor(
                    rstg[:], Bp[:], 1.0 / S8,
                    as_g[:, h * HB : (h + 1) * HB, 0:DM],
                    op0=ALU.mult, op1=ALU.add,
                )
                # exp over the whole half-block in one ACT op
                hsl = slice(h * HB, (h + 1) * HB)
                nc.scalar.activation(
                    ex_g[:, hsl, :], rstg[:], AF.Exp, bias=bias_shift[:],
                )
                # heavy (720-col) tail ops at half-block granularity so only
                # a light suffix trails the last matmul
                nc.vector.reduce_sum(
                    sums[:, hsl, :],
                    ex_g[:, hsl, :].rearrange("p s (d m) -> p s d m", d=D, m=M),
                    axis=mybir.AxisListType.X,
                )
                prod = stg.tile([P, HB, DM], F32, tag="prod")
                nc.vector.tensor_mul(
                    prod[:], ex_g[:, hsl, :], as_g[:, hsl, DM : 2 * DM]
                )
                nc.vector.reduce_sum(
                    wsum[:, hsl, :],
                    prod[:].rearrange("p s (d m) -> p s d m", d=D, m=M),
                    axis=mybir.AxisListType.X,
                )

            # Light per-group softmax suffix
            rsums = spool.tile([P, G, D], F32, tag="rsums")
            nc.vector.reciprocal(rsums[:], sums[:])
            datt = spool.tile([P, G, D], F32, tag="datt")
            nc.vector.tensor_mul(datt[:], wsum[:], rsums[:])
            ex2 = spool.tile([P, G, D], F32, tag="ex2")
            nc.scalar.activation(ex2[:], datt[:], AF.Exp, bias=bias_shift[:])
            sumd = spool.tile([P, G], F32, tag="sumd")
            nc.vector.reduce_sum(sumd[:], ex2[:], axis=mybir.AxisListType.X)
            rd = spool.tile([P, G], F32, tag="rd")
            nc.vector.reciprocal(rd[:], sumd[:])
            nc.vector.tensor_mul(
                out_sb[:, g * G : (g + 1) * G, :],
                ex2[:],
                rd[:, :, None].broadcast_to([P, G, D]),
            )
            # Output rides the gpsimd queue (last group: sync, idle by then)
            eng = nc.sync if g == NG - 1 else nc.gpsimd
            eng.dma_start(
                out[:, g * G : (g + 1) * G], out_sb[:, g * G : (g + 1) * G, :]
            )

    # Keep Exp + Copy in one ACT table set to avoid ~2.7us table swaps.
    mine = {AF.Exp, AF.Ln, AF.Square, AF.Copy, AF.Identity}
    orig_tables = bacc.get_activation_tables

    def _patched(arch):
        return {
            name: (fns if name == "natural_log_exp_and_others" else fns - mine)
            for name, fns in orig_tables(arch).items()
        }

    bacc.get_activation_tables = _patched
    try:
        nc.finalize()
    finally:
        bacc.get_activation_tables = orig_tables
    return nc


def _get_nc():
    if "nc" not in _CACHE:
        _CACHE["nc"] = _build_nc()
    return _CACHE["nc"]


def _host_prep(feature, W_topic, W_domain, memory):
    """Fold memory+TAU into the projections; split planes; pre-tile layouts."""
    import ml_dtypes

    E4 = ml_dtypes.float8_e4m3

    mem_flat = memory.reshape(D * M, E).astype(np.float64)
    RS = TAU * (mem_flat @ W_topic.astype(np.float64))   # [90, 768]
    RT = TAU * (mem_flat @ W_domain.astype(np.float64))  # [90, 768]
    RhiS = RS.astype(np.float16)
    RloS = (RS - RhiS.astype(np.float64)).astype(np.float16)
    RhiT = RT.astype(np.float16)
    rtcat = np.concatenate([RhiS.T, RhiT.T, RloS.T], axis=1)  # [768, 270]
    rt = np.ascontiguousarray(
        rtcat.reshape(KC, P, 3 * DM).transpose(1, 0, 2)
    )  # [P, KC, 270]
    rt8 = np.ascontiguousarray(
        RS.astype(E4).T.reshape(KC, P, DM).transpose(1, 0, 2)
    )  # [P, KC, 90]

    f = np.asarray(feature, dtype=np.float32)
    fn = f / np.sqrt((f.astype(np.float64) ** 2).sum(axis=1, keepdims=True)).astype(
        np.float32
    )

    per_core = []
    for c in range(NCORES):
        fc = fn[c * BC : (c + 1) * BC]  # [4096, 768]
        fhi = fc.astype(np.float16)
        flo = (fc - fhi.astype(np.float32)) * np.float32(S8)
        # [p, t, k, cc] = fc[t*128+cc, k*128+p]
        fhi_t = np.ascontiguousarray(
            fhi.reshape(NT, P, KC, P).transpose(3, 0, 2, 1)
        )
        flo8_t = np.ascontiguousarray(
            flo.astype(E4).reshape(NT, P, KC, P).transpose(3, 0, 2, 1)
        )
        per_core.append({"fhi": fhi_t, "flo8": flo8_t, "rt": rt, "rt8": rt8})
    return per_core


def kernel(feature, category, W_topic, W_domain, memory):
    from concourse.bass_utils import run_bass_kernel_spmd

    in_maps = _host_prep(
        feature, np.asarray(W_topic), np.asarray(W_domain), np.asarray(memory)
    )
    nc = _get_nc()
    res = run_bass_kernel_spmd(nc, in_maps, core_ids=list(range(NCORES)))
    outs = [
        res.results[c]["out"].transpose(1, 0, 2).reshape(BC, D)
        for c in range(NCORES)
    ]
    full = np.concatenate(outs, axis=0)  # [B, 9]
    return full[:, None, :].astype(np.float32)


# revision 23
# speedup vs baseline: 1.1760x; 1.1760x over previous
"""Trainium2 Bass kernel for nn_MemoryNetwork (scatter_memory).

Computation (reference, per batch row b):
    f = feature / ||feature||                       [B, 768]
    topic = f @ W_topic.T ; dom = f @ W_domain.T    [B, 256]
    att   = softmax_m(TAU * topic . memory[d,m])    [B, 9, 10]
    sep   = sum_m att * memory[d,m]                 [B, 9, 256]
    out   = softmax_d(TAU * sep . dom)              [B, 1, 9]

Reformulation: fold the tiny memory banks and TAU into the projections on
the host (f normalized host-side too, so no per-row scale on device):
    RS = TAU * mem_flat @ W_topic   [90, 768]
    RT = TAU * mem_flat @ W_domain  [90, 768]
    rawS = fn @ RS.T ; rawT = fn @ RT.T             [B, 90] each
    ex   = exp(rawS - SHIFT)          (softmax_m numerator; logits in
                                       [-123, 105] so a const shift is safe)
    datt = (sum_m ex*rawT) / (sum_m ex)
    out  = softmax_d(datt)            (const shift again)

Precision: rawS feeds an exponent with TAU-amplified spread, so it needs
~2^-15 relative accuracy on f; rawT only enters linearly and tolerates
plain fp16. Scheme (validated host-side, rel err 7.0e-3 vs 2e-2 gate):
    fhi  = fp16(fn);  flo8 = e4m3((fn - fhi) * 2^17)   (DMA: 3 B/elem)
    RhiS/RloS = fp16 split of RS;  RhiT = fp16(RT);  RS8 = e4m3(RS)
    rawS = fhi@RhiS + fhi@RloS + (flo8@RS8) * 2^-17;  rawT = fhi@RhiT
Per 128-contraction chunk that is 3 PE matmuls: 180-col fp16 [RhiS|RhiT],
90-col fp16 RloS (PSUM-accumulated onto the S columns), 90-col fp8 into a
separate bank (carries the 2^17 scale). LDWEIGHTS fully hides under the
streams (measured), so the PE floor is ~360 cols/chunk = 150 ns warm.

Sharding: data-parallel over B across 8 cores (4096 rows each). All DRAM
layouts are pre-tiled host-side so every DMA descriptor is >=3 KB
contiguous per partition. A burst of dependency-free warmup matmuls at
t=0 starts the PE HAM clock ramp (1.2 -> 2.4 GHz) while DMA fills.
"""

import sys

sys.path.insert(0, "/opt/trn_rl_repo")

import numpy as np

B, IN, E, D, M = 32768, 768, 256, 9, 10
NCORES = 8
BC = B // NCORES  # rows per core
P = 128           # partition tile
NT = BC // P      # batch tiles per core (32)
G = 8             # tiles per softmax group
NG = NT // G      # groups (4)
HB = 4            # tiles per DMA half-block
KC = IN // P      # contraction chunks (6)
DM = D * M        # 90
TAU = 32.0
SHIFT = 50.0
S8 = 2.0 ** 17    # flo8 pre-scale
N_WARM = 40       # HAM warmup matmuls (bridge the DMA fill, trip the ramp)

_CACHE: dict = {}


def _build_nc(repeat=1):
    from contextlib import ExitStack

    import concourse.bacc as bacc
    import concourse.tile as tile
    from concourse import mybir

    F32 = mybir.dt.float32
    F16 = mybir.dt.float16
    F8 = mybir.dt.float8e4
    AF = mybir.ActivationFunctionType
    ALU = mybir.AluOpType

    nc = bacc.Bacc(trn_type="TRN2")
    fhi = nc.dram_tensor("fhi", [P, NT, KC, P], F16, kind="ExternalInput")
    flo8 = nc.dram_tensor("flo8", [P, NT, KC, P], F8, kind="ExternalInput")
    # rt columns: 0:90 RhiS, 90:180 RhiT, 180:270 RloS
    rt = nc.dram_tensor("rt", [P, KC, 3 * DM], F16, kind="ExternalInput")
    rt8 = nc.dram_tensor("rt8", [P, KC, DM], F8, kind="ExternalInput")
    out = nc.dram_tensor("out", [P, NT, D], F32, kind="ExternalOutput")

    with tile.TileContext(nc) as tc, ExitStack() as ctx:
        const = ctx.enter_context(tc.tile_pool(name="const", bufs=1))
        # All feature planes fit SBUF (9.4 MB) — buffer everything and let
        # the DMA rings run gap-free instead of self-pacing behind the PE.
        fpool = ctx.enter_context(tc.tile_pool(name="fts", bufs=1))
        lpool = ctx.enter_context(tc.tile_pool(name="lts", bufs=1))
        stg = ctx.enter_context(tc.tile_pool(name="stg", bufs=2))
        gpool = ctx.enter_context(tc.tile_pool(name="grp", bufs=2))
        spool = ctx.enter_context(tc.tile_pool(name="small", bufs=2))
        apool = ctx.enter_context(tc.tile_pool(name="aps", bufs=4, space="PSUM"))
        bpool = ctx.enter_context(tc.tile_pool(name="bps", bufs=2, space="PSUM"))
        wpool = ctx.enter_context(tc.tile_pool(name="wps", bufs=1, space="PSUM"))

        rt_sb = const.tile([P, KC, 3 * DM], F16)
        rt8_sb = const.tile([P, KC, DM], F8)
        bias_shift = const.tile([P, 1], F32)
        out_sb = const.tile([P, NT, D], F32)
        wz = const.tile([P, P], F16)

        # Feature tiles: everything resident. Group 0 split per half-block
        # (and per tile on the wire) so the first matmuls start early;
        # groups 1-3 are single full-group transfers.
        f0 = fpool.tile([P, HB, KC, P], F16, tag="f0")
        f1 = fpool.tile([P, HB, KC, P], F16, tag="f1")
        l0 = lpool.tile([P, HB, KC, P], F8, tag="l0")
        l1 = lpool.tile([P, HB, KC, P], F8, tag="l1")
        fgs = [
            fpool.tile([P, G, KC, P], F16, tag=f"fg{i}", name=f"fg{i}")
            for i in (1, 2, 3)
        ]
        lgs = [
            lpool.tile([P, G, KC, P], F8, tag=f"lg{i}", name=f"lg{i}")
            for i in (1, 2, 3)
        ]

        def ftile(t):
            if t < HB:
                return f0, t
            if t < G:
                return f1, t - HB
            return fgs[t // G - 1], t % G

        def ltile(t):
            if t < HB:
                return l0, t
            if t < G:
                return l1, t - HB
            return lgs[t // G - 1], t % G

        # Memsets on the vector engine: warmup needs wz immediately and the
        # gpsimd queue must stay pure DMA.
        nc.vector.memset(wz[:], 0.0)
        nc.vector.memset(bias_shift[:], -SHIFT)

        # The kernel is DMA-throughput-bound (~270 GB/s effective/core vs
        # a 240 GB/s steady PE demand), so the stream must be dense and
        # need-ordered from the start. Issue everything up front in need
        # order, greedily balancing bytes across the two rings (sync HWDGE
        # + gpsimd SWDGE; nc.scalar would steal ACT time).
        # Uniform half-block transfer quanta: a tile becomes usable only
        # when its whole transfer lands, and the stream (~270 GB/s) is
        # slower than PE demand (~330), so large quanta starve the PE and
        # re-throttle the HAM clock. Group 0 goes per-tile for early start.
        FB = KC * P * 2          # fp16 bytes per feature tile per partition
        xfers = []               # (bytes, dst_ap, src_ap) in need order
        xfers.append((P * 2 * 270 * 2, rt_sb[:, 0:2, :], rt[:, 0:2, :]))
        xfers.append((P * KC * DM, rt8_sb[:], rt8[:, :, :]))
        for t in range(HB):
            xfers.append((P * FB, f0[:, t : t + 1], fhi[:, t : t + 1]))
            xfers.append((P * FB // 2, l0[:, t : t + 1], flo8[:, t : t + 1]))
            if t < 2:
                sl = slice(2 + 2 * t, 4 + 2 * t)
                xfers.append((P * 2 * 270 * 2, rt_sb[:, sl, :], rt[:, sl, :]))
        xfers.append((P * HB * FB, f1[:], fhi[:, HB : 2 * HB]))
        xfers.append((P * HB * FB // 2, l1[:], flo8[:, HB : 2 * HB]))
        for i in (1, 2, 3):
            for h in range(2):
                hsl = slice(h * HB, (h + 1) * HB)
                dsl = slice(i * G + h * HB, i * G + (h + 1) * HB)
                xfers.append((P * HB * FB, fgs[i - 1][:, hsl], fhi[:, dsl]))
                xfers.append((P * HB * FB // 2, lgs[i - 1][:, hsl],
                              flo8[:, dsl]))

        cum = [0, 0]
        engs = [nc.sync, nc.gpsimd]
        for nbytes, dst, src in xfers:
            ring = 0 if cum[0] <= cum[1] else 1
            engs[ring].dma_start(dst, src)
            cum[ring] += nbytes

        # HAM warmup: dependency-free matmuls keep the PE busy from t~0 so
        # the 2.4 GHz un-throttle fires while the first feature DMA lands.
        wps = wpool.tile([P, DM], F32)
        for _ in range(N_WARM):
            nc.tensor.matmul(wps[:], wz[:], wz[:, 0:DM], start=True, stop=True)

        def emit_suffix(g, tsl):
            """Softmax-over-domains suffix for group-tile slice tsl."""
            rsums = spool.tile([P, G, D], F32, tag="rsums", name="rsums")
            nc.vector.reciprocal(rsums[:, tsl, :], sums[:, tsl, :])
            datt = spool.tile([P, G, D], F32, tag="datt", name="datt")
            nc.vector.tensor_mul(datt[:, tsl, :], wsum[:, tsl, :], rsums[:, tsl, :])
            ex2 = spool.tile([P, G, D], F32, tag="ex2", name="ex2")
            nc.scalar.activation(
                ex2[:, tsl, :], datt[:, tsl, :], AF.Exp, bias=bias_shift[:]
            )
            sumd = spool.tile([P, G], F32, tag="sumd", name="sumd")
            nc.vector.reduce_sum(
                sumd[:, tsl], ex2[:, tsl, :], axis=mybir.AxisListType.X
            )
            rd = spool.tile([P, G], F32, tag="rd", name="rd")
            nc.vector.reciprocal(rd[:, tsl], sumd[:, tsl])
            gsl = slice(g * G + tsl.start, g * G + tsl.stop)
            nc.vector.tensor_mul(
                out_sb[:, gsl, :],
                ex2[:, tsl, :],
                rd[:, tsl, None].broadcast_to(
                    [P, tsl.stop - tsl.start, D]
                ),
            )
            return gsl

        pending_out = None
        for gi in range(NG * repeat):
            g = gi % NG
            is_last_g = gi == NG * repeat - 1
            ex_g = gpool.tile([P, G, DM], F32, tag="exg")
            # staged [rawS_main | rawT] per tile, copied from PSUM by ACT
            as_g = gpool.tile([P, G, 2 * DM], F32, tag="asg")
            sums = spool.tile([P, G, D], F32, tag="sums")
            wsum = spool.tile([P, G, D], F32, tag="wsum")

            for h in range(G // HB):
                hb = g * (G // HB) + h
                rstg = stg.tile([P, HB, DM], F32, tag="rstg")
                # fp8 corrections for all HB tiles share one PSUM bank
                Bp = bpool.tile([P, HB, DM], F32, tag="B")

                for s in range(HB):
                    sg = h * HB + s  # tile index within group
                    t = hb * HB + s
                    fh, fi = ftile(t)
                    lh, li = ltile(t)
                    A = apool.tile([P, 2 * DM], F32, tag="A")
                    for k in range(KC):
                        # A[:, 0:90] += fhi@RhiS ; A[:, 90:180] += fhi@RhiT
                        nc.tensor.matmul(
                            A[:], fh[:, fi, k, :], rt_sb[:, k, 0 : 2 * DM],
                            start=(k == 0), stop=False,
                        )
                        # A[:, 0:90] += fhi@RloS  (same-column accumulate)
                        nc.tensor.matmul(
                            A[:, 0:DM], fh[:, fi, k, :],
                            rt_sb[:, k, 2 * DM : 3 * DM],
                            start=False, stop=(k == KC - 1),
                            skip_group_check=True,
                        )
                        # Bp[:, s] += flo8@RS8   (scaled by 2^17)
                        nc.tensor.matmul(
                            Bp[:, s, :], lh[:, li, k, :], rt8_sb[:, k, :],
                            start=(k == 0), stop=(k == KC - 1),
                        )
                    # stage [rawS_main | rawT] to SBUF (frees the PSUM bank)
                    nc.scalar.copy(as_g[:, sg, :], A[:])

                # one fused rawS = Bp * 2^-17 + A_S per half-block
                nc.vector.scalar_tensor_tensor(
                    rstg[:], Bp[:], 1.0 / S8,
                    as_g[:, h * HB : (h + 1) * HB, 0:DM],
                    op0=ALU.mult, op1=ALU.add,
                )
                # exp over the whole half-block in one ACT op
                hsl = slice(h * HB, (h + 1) * HB)
                nc.scalar.activation(
                    ex_g[:, hsl, :], rstg[:], AF.Exp, bias=bias_shift[:],
                )
                # heavy (720-col) tail ops at half-block granularity so only
                # a light suffix trails the last matmul
                nc.vector.reduce_sum(
                    sums[:, hsl, :],
                    ex_g[:, hsl, :].rearrange("p s (d m) -> p s d m", d=D, m=M),
                    axis=mybir.AxisListType.X,
                )
                prod = stg.tile([P, HB, DM], F32, tag="prod")
                nc.vector.tensor_mul(
                    prod[:], ex_g[:, hsl, :], as_g[:, hsl, DM : 2 * DM]
                )
                nc.vector.reduce_sum(
                    wsum[:, hsl, :],
                    prod[:].rearrange("p s (d m) -> p s d m", d=D, m=M),
                    axis=mybir.AxisListType.X,
                )

                if h == 0 and pending_out is not None:
                    # previous group's output: issue from the ACT ring well
                    # after its producer finished (no FIFO stall), keeping
                    # the feature rings pure
                    nc.scalar.dma_start(
                        out[:, pending_out], out_sb[:, pending_out, :]
                    )
                    pending_out = None
                if is_last_g:
                    # last group: suffix + output per half-block so only a
                    # short chain trails the final matmul
                    gsl = emit_suffix(g, hsl)
                    nc.sync.dma_start(out[:, gsl], out_sb[:, gsl, :])

            if not is_last_g:
                pending_out = emit_suffix(g, slice(0, G))

    # Keep Exp + Copy in one ACT table set to avoid ~2.7us table swaps.
    mine = {AF.Exp, AF.Ln, AF.Square, AF.Copy, AF.Identity}
    orig_tables = bacc.get_activation_tables

    def _patched(arch):
        return {
            name: (fns if name == "natural_log_exp_and_others" else fns - mine)
            for name, fns in orig_tables(arch).items()
        }

    bacc.get_activation_tables = _patched
    try:
        nc.finalize()
    finally:
        bacc.get_activation_tables = orig_tables
    return nc


def _get_nc():
    if "nc" not in _CACHE:
        _CACHE["nc"] = _build_nc()
    return _CACHE["nc"]


def _host_prep(feature, W_topic, W_domain, memory):
    """Fold memory+TAU into the projections; split planes; pre-tile layouts."""
    import ml_dtypes

    E4 = ml_dtypes.float8_e4m3

    mem_flat = memory.reshape(D * M, E).astype(np.float64)
    RS = TAU * (mem_flat @ W_topic.astype(np.float64))   # [90, 768]
    RT = TAU * (mem_flat @ W_domain.astype(np.float64))  # [90, 768]
    RhiS = RS.astype(np.float16)
    RloS = (RS - RhiS.astype(np.float64)).astype(np.float16)
    RhiT = RT.astype(np.float16)
    rtcat = np.concatenate([RhiS.T, RhiT.T, RloS.T], axis=1)  # [768, 270]
    rt = np.ascontiguousarray(
        rtcat.reshape(KC, P, 3 * DM).transpose(1, 0, 2)
    )  # [P, KC, 270]
    rt8 = np.ascontiguousarray(
        RS.astype(E4).T.reshape(KC, P, DM).transpose(1, 0, 2)
    )  # [P, KC, 90]

    f = np.asarray(feature, dtype=np.float32)
    fn = f / np.sqrt((f.astype(np.float64) ** 2).sum(axis=1, keepdims=True)).astype(
        np.float32
    )

    per_core = []
    for c in range(NCORES):
        fc = fn[c * BC : (c + 1) * BC]  # [4096, 768]
        fhi = fc.astype(np.float16)
        flo = (fc - fhi.astype(np.float32)) * np.float32(S8)
        # [p, t, k, cc] = fc[t*128+cc, k*128+p]
        fhi_t = np.ascontiguousarray(
            fhi.reshape(NT, P, KC, P).transpose(3, 0, 2, 1)
        )
        flo8_t = np.ascontiguousarray(
            flo.astype(E4).reshape(NT, P, KC, P).transpose(3, 0, 2, 1)
        )
        per_core.append({"fhi": fhi_t, "flo8": flo8_t, "rt": rt, "rt8": rt8})
    return per_core


def kernel(feature, category, W_topic, W_domain, memory):
    from concourse.bass_utils import run_bass_kernel_spmd

    in_maps = _host_prep(
        feature, np.asarray(W_topic), np.asarray(W_domain), np.asarray(memory)
    )
    nc = _get_nc()
    res = run_bass_kernel_spmd(nc, in_maps, core_ids=list(range(NCORES)))
    outs = [
        res.results[c]["out"].transpose(1, 0, 2).reshape(BC, D)
        for c in range(NCORES)
    ]
    full = np.concatenate(outs, axis=0)  # [B, 9]
    return full[:, None, :].astype(np.float32)


# revision 25
# speedup vs baseline: 1.1789x; 1.0025x over previous
"""Trainium2 Bass kernel for nn_MemoryNetwork (scatter_memory).

Computation (reference, per batch row b):
    f = feature / ||feature||                       [B, 768]
    topic = f @ W_topic.T ; dom = f @ W_domain.T    [B, 256]
    att   = softmax_m(TAU * topic . memory[d,m])    [B, 9, 10]
    sep   = sum_m att * memory[d,m]                 [B, 9, 256]
    out   = softmax_d(TAU * sep . dom)              [B, 1, 9]

Reformulation: fold the tiny memory banks and TAU into the projections on
the host (f normalized host-side too, so no per-row scale on device):
    RS = TAU * mem_flat @ W_topic   [90, 768]
    RT = TAU * mem_flat @ W_domain  [90, 768]
    rawS = fn @ RS.T ; rawT = fn @ RT.T             [B, 90] each
    ex   = exp(rawS - SHIFT)          (softmax_m numerator; logits in
                                       [-123, 105] so a const shift is safe)
    datt = (sum_m ex*rawT) / (sum_m ex)
    out  = softmax_d(datt)            (const shift again)

Precision: rawS feeds an exponent with TAU-amplified spread, so it needs
~2^-15 relative accuracy on f; rawT only enters linearly and tolerates
plain fp16. Scheme (validated host-side, rel err 7.0e-3 vs 2e-2 gate):
    fhi  = fp16(fn);  flo8 = e4m3((fn - fhi) * 2^17)   (DMA: 3 B/elem)
    RhiS/RloS = fp16 split of RS;  RhiT = fp16(RT);  RS8 = e4m3(RS)
    rawS = fhi@RhiS + fhi@RloS + (flo8@RS8) * 2^-17;  rawT = fhi@RhiT
Per 128-contraction chunk that is 3 PE matmuls: 180-col fp16 [RhiS|RhiT],
90-col fp16 RloS (PSUM-accumulated onto the S columns), 90-col fp8 into a
separate bank (carries the 2^17 scale). LDWEIGHTS fully hides under the
streams (measured), so the PE floor is ~360 cols/chunk = 150 ns warm.

Sharding: data-parallel over B across 8 cores (4096 rows each). All DRAM
layouts are pre-tiled host-side so every DMA descriptor is >=3 KB
contiguous per partition. A burst of dependency-free warmup matmuls at
t=0 starts the PE HAM clock ramp (1.2 -> 2.4 GHz) while DMA fills.
"""

import sys

sys.path.insert(0, "/opt/trn_rl_repo")

import numpy as np

B, IN, E, D, M = 32768, 768, 256, 9, 10
NCORES = 8
BC = B // NCORES  # rows per core
P = 128           # partition tile
NT = BC // P      # batch tiles per core (32)
G = 8             # tiles per softmax group
NG = NT // G      # groups (4)
HB = 4            # tiles per DMA half-block
KC = IN // P      # contraction chunks (6)
DM = D * M        # 90
TAU = 32.0
SHIFT = 50.0
S8 = 2.0 ** 17    # flo8 pre-scale
N_WARM = 24       # HAM warmup matmuls (bridge the DMA fill, trip the ramp)

_CACHE: dict = {}


def _build_nc(repeat=1):
    from contextlib import ExitStack

    import concourse.bacc as bacc
    import concourse.tile as tile
    from concourse import mybir

    F32 = mybir.dt.float32
    F16 = mybir.dt.float16
    F8 = mybir.dt.float8e4
    AF = mybir.ActivationFunctionType
    ALU = mybir.AluOpType

    nc = bacc.Bacc(trn_type="TRN2")
    fhi = nc.dram_tensor("fhi", [P, NT, KC, P], F16, kind="ExternalInput")
    flo8 = nc.dram_tensor("flo8", [P, NT, KC, P], F8, kind="ExternalInput")
    # rt columns: 0:90 RhiS, 90:180 RhiT, 180:270 RloS
    rt = nc.dram_tensor("rt", [P, KC, 3 * DM], F16, kind="ExternalInput")
    rt8 = nc.dram_tensor("rt8", [P, KC, DM], F8, kind="ExternalInput")
    out = nc.dram_tensor("out", [P, NT, D], F32, kind="ExternalOutput")

    with tile.TileContext(nc) as tc, ExitStack() as ctx:
        const = ctx.enter_context(tc.tile_pool(name="const", bufs=1))
        # All feature planes fit SBUF (9.4 MB) — buffer everything and let
        # the DMA rings run gap-free instead of self-pacing behind the PE.
        fpool = ctx.enter_context(tc.tile_pool(name="fts", bufs=1))
        lpool = ctx.enter_context(tc.tile_pool(name="lts", bufs=1))
        stg = ctx.enter_context(tc.tile_pool(name="stg", bufs=2))
        gpool = ctx.enter_context(tc.tile_pool(name="grp", bufs=2))
        spool = ctx.enter_context(tc.tile_pool(name="small", bufs=2))
        apool = ctx.enter_context(tc.tile_pool(name="aps", bufs=4, space="PSUM"))
        bpool = ctx.enter_context(tc.tile_pool(name="bps", bufs=2, space="PSUM"))
        wpool = ctx.enter_context(tc.tile_pool(name="wps", bufs=1, space="PSUM"))

        rt_sb = const.tile([P, KC, 3 * DM], F16)
        rt8_sb = const.tile([P, KC, DM], F8)
        bias_shift = const.tile([P, 1], F32)
        out_sb = const.tile([P, NT, D], F32)
        wz = const.tile([P, P], F16)

        # Feature tiles: everything resident. Group 0 split per half-block
        # (and per tile on the wire) so the first matmuls start early;
        # groups 1-3 are single full-group transfers.
        f0 = fpool.tile([P, HB, KC, P], F16, tag="f0")
        f1 = fpool.tile([P, HB, KC, P], F16, tag="f1")
        l0 = lpool.tile([P, HB, KC, P], F8, tag="l0")
        l1 = lpool.tile([P, HB, KC, P], F8, tag="l1")
        fgs = [
            fpool.tile([P, G, KC, P], F16, tag=f"fg{i}", name=f"fg{i}")
            for i in (1, 2, 3)
        ]
        lgs = [
            lpool.tile([P, G, KC, P], F8, tag=f"lg{i}", name=f"lg{i}")
            for i in (1, 2, 3)
        ]

        def ftile(t):
            if t < HB:
                return f0, t
            if t < G:
                return f1, t - HB
            return fgs[t // G - 1], t % G

        def ltile(t):
            if t < HB:
                return l0, t
            if t < G:
                return l1, t - HB
            return lgs[t // G - 1], t % G

        # Memsets on the vector engine: warmup needs wz immediately and the
        # gpsimd queue must stay pure DMA.
        nc.vector.memset(wz[:], 0.0)
        nc.vector.memset(bias_shift[:], -SHIFT)

        # The kernel is DMA-throughput-bound (~270 GB/s effective/core vs
        # a 240 GB/s steady PE demand), so the stream must be dense and
        # need-ordered from the start. Issue everything up front in need
        # order, greedily balancing bytes across the two rings (sync HWDGE
        # + gpsimd SWDGE; nc.scalar would steal ACT time).
        # Uniform half-block transfer quanta: a tile becomes usable only
        # when its whole transfer lands, and the stream (~270 GB/s) is
        # slower than PE demand (~330), so large quanta starve the PE and
        # re-throttle the HAM clock. Group 0 goes per-tile for early start.
        FB = KC * P * 2          # fp16 bytes per feature tile per partition
        xfers = []               # (bytes, dst_ap, src_ap) in need order
        xfers.append((P * 2 * 270 * 2, rt_sb[:, 0:2, :], rt[:, 0:2, :]))
        xfers.append((P * KC * DM, rt8_sb[:], rt8[:, :, :]))
        for t in range(HB):
            xfers.append((P * FB, f0[:, t : t + 1], fhi[:, t : t + 1]))
            xfers.append((P * FB // 2, l0[:, t : t + 1], flo8[:, t : t + 1]))
            if t < 2:
                sl = slice(2 + 2 * t, 4 + 2 * t)
                xfers.append((P * 2 * 270 * 2, rt_sb[:, sl, :], rt[:, sl, :]))
        xfers.append((P * HB * FB, f1[:], fhi[:, HB : 2 * HB]))
        xfers.append((P * HB * FB // 2, l1[:], flo8[:, HB : 2 * HB]))
        for i in (1, 2, 3):
            for h in range(2):
                hsl = slice(h * HB, (h + 1) * HB)
                dsl = slice(i * G + h * HB, i * G + (h + 1) * HB)
                xfers.append((P * HB * FB, fgs[i - 1][:, hsl], fhi[:, dsl]))
                xfers.append((P * HB * FB // 2, lgs[i - 1][:, hsl],
                              flo8[:, dsl]))

        cum = [0, 0]
        engs = [nc.sync, nc.gpsimd]
        # Tile 0's planes ride HWDGE (sync): SWDGE completion costs ~2us
        # extra, which would sit on the first-matmul critical path.
        pinned = {0: 0, 1: 1, 2: 0, 3: 0}  # xfer idx -> ring
        for i, (nbytes, dst, src) in enumerate(xfers):
            ring = pinned.get(i, 0 if cum[0] <= cum[1] else 1)
            engs[ring].dma_start(dst, src)
            cum[ring] += nbytes

        # HAM warmup: dependency-free matmuls keep the PE busy from t~0 so
        # the 2.4 GHz un-throttle fires while the first feature DMA lands.
        wps = wpool.tile([P, DM], F32)
        for _ in range(N_WARM):
            nc.tensor.matmul(wps[:], wz[:], wz[:, 0:DM], start=True, stop=True)

        def emit_suffix(g, tsl):
            """Softmax-over-domains suffix for group-tile slice tsl."""
            rsums = spool.tile([P, G, D], F32, tag="rsums", name="rsums")
            nc.vector.reciprocal(rsums[:, tsl, :], sums[:, tsl, :])
            datt = spool.tile([P, G, D], F32, tag="datt", name="datt")
            nc.vector.tensor_mul(datt[:, tsl, :], wsum[:, tsl, :], rsums[:, tsl, :])
            ex2 = spool.tile([P, G, D], F32, tag="ex2", name="ex2")
            nc.scalar.activation(
                ex2[:, tsl, :], datt[:, tsl, :], AF.Exp, bias=bias_shift[:]
            )
            sumd = spool.tile([P, G], F32, tag="sumd", name="sumd")
            nc.vector.reduce_sum(
                sumd[:, tsl], ex2[:, tsl, :], axis=mybir.AxisListType.X
            )
            rd = spool.tile([P, G], F32, tag="rd", name="rd")
            nc.vector.reciprocal(rd[:, tsl], sumd[:, tsl])
            gsl = slice(g * G + tsl.start, g * G + tsl.stop)
            nc.vector.tensor_mul(
                out_sb[:, gsl, :],
                ex2[:, tsl, :],
                rd[:, tsl, None].broadcast_to(
                    [P, tsl.stop - tsl.start, D]
                ),
            )
            return gsl

        pending_out = None
        for gi in range(NG * repeat):
            g = gi % NG
            is_last_g = gi == NG * repeat - 1
            ex_g = gpool.tile([P, G, DM], F32, tag="exg")
            # staged [rawS_main | rawT] per tile, copied from PSUM by ACT
            as_g = gpool.tile([P, G, 2 * DM], F32, tag="asg")
            sums = spool.tile([P, G, D], F32, tag="sums")
            wsum = spool.tile([P, G, D], F32, tag="wsum")

            for h in range(G // HB):
                hb = g * (G // HB) + h
                rstg = stg.tile([P, HB, DM], F32, tag="rstg")
                # fp8 corrections for all HB tiles share one PSUM bank
                Bp = bpool.tile([P, HB, DM], F32, tag="B")

                for s in range(HB):
                    sg = h * HB + s  # tile index within group
                    t = hb * HB + s
                    fh, fi = ftile(t)
                    lh, li = ltile(t)
                    A = apool.tile([P, 2 * DM], F32, tag="A")
                    for k in range(KC):
                        # A[:, 0:90] += fhi@RhiS ; A[:, 90:180] += fhi@RhiT
                        nc.tensor.matmul(
                            A[:], fh[:, fi, k, :], rt_sb[:, k, 0 : 2 * DM],
                            start=(k == 0), stop=False,
                        )
                        # A[:, 0:90] += fhi@RloS  (same-column accumulate)
                        nc.tensor.matmul(
                            A[:, 0:DM], fh[:, fi, k, :],
                            rt_sb[:, k, 2 * DM : 3 * DM],
                            start=False, stop=(k == KC - 1),
                            skip_group_check=True,
                        )
                        # Bp[:, s] += flo8@RS8   (scaled by 2^17)
                        nc.tensor.matmul(
                            Bp[:, s, :], lh[:, li, k, :], rt8_sb[:, k, :],
                            start=(k == 0), stop=(k == KC - 1),
                        )
                    # stage [rawS_main | rawT] to SBUF (frees the PSUM bank)
                    nc.scalar.copy(as_g[:, sg, :], A[:])

                # one fused rawS = Bp * 2^-17 + A_S per half-block
                nc.vector.scalar_tensor_tensor(
                    rstg[:], Bp[:], 1.0 / S8,
                    as_g[:, h * HB : (h + 1) * HB, 0:DM],
                    op0=ALU.mult, op1=ALU.add,
                )
                # exp over the whole half-block in one ACT op
                hsl = slice(h * HB, (h + 1) * HB)
                nc.scalar.activation(
                    ex_g[:, hsl, :], rstg[:], AF.Exp, bias=bias_shift[:],
                )
                # heavy (720-col) tail ops at half-block granularity so only
                # a light suffix trails the last matmul
                nc.vector.reduce_sum(
                    sums[:, hsl, :],
                    ex_g[:, hsl, :].rearrange("p s (d m) -> p s d m", d=D, m=M),
                    axis=mybir.AxisListType.X,
                )
                prod = stg.tile([P, HB, DM], F32, tag="prod")
                nc.vector.tensor_mul(
                    prod[:], ex_g[:, hsl, :], as_g[:, hsl, DM : 2 * DM]
                )
                nc.vector.reduce_sum(
                    wsum[:, hsl, :],
                    prod[:].rearrange("p s (d m) -> p s d m", d=D, m=M),
                    axis=mybir.AxisListType.X,
                )

                if h == 0 and pending_out is not None:
                    # previous group's output: issue from the ACT ring well
                    # after its producer finished (no FIFO stall), keeping
                    # the feature rings pure
                    nc.scalar.dma_start(
                        out[:, pending_out], out_sb[:, pending_out, :]
                    )
                    pending_out = None
                if is_last_g:
                    # last group: suffix + output per half-block so only a
                    # short chain trails the final matmul
                    gsl = emit_suffix(g, hsl)
                    nc.sync.dma_start(out[:, gsl], out_sb[:, gsl, :])

            if not is_last_g:
                pending_out = emit_suffix(g, slice(0, G))

    # Keep Exp + Copy in one ACT table set to avoid ~2.7us table swaps.
    mine = {AF.Exp, AF.Ln, AF.Square, AF.Copy, AF.Identity}
    orig_tables = bacc.get_activation_tables

    def _patched(arch):
        return {
            name: (fns if name == "natural_log_exp_and_others" else fns - mine)
            for name, fns in orig_tables(arch).items()
        }

    bacc.get_activation_tables = _patched
    try:
        nc.finalize()
    finally:
        bacc.get_activation_tables = orig_tables
    return nc


def _get_nc():
    if "nc" not in _CACHE:
        _CACHE["nc"] = _build_nc()
    return _CACHE["nc"]


def _host_prep(feature, W_topic, W_domain, memory):
    """Fold memory+TAU into the projections; split planes; pre-tile layouts."""
    import ml_dtypes

    E4 = ml_dtypes.float8_e4m3

    mem_flat = memory.reshape(D * M, E).astype(np.float64)
    RS = TAU * (mem_flat @ W_topic.astype(np.float64))   # [90, 768]
    RT = TAU * (mem_flat @ W_domain.astype(np.float64))  # [90, 768]
    RhiS = RS.astype(np.float16)
    RloS = (RS - RhiS.astype(np.float64)).astype(np.float16)
    RhiT = RT.astype(np.float16)
    rtcat = np.concatenate([RhiS.T, RhiT.T, RloS.T], axis=1)  # [768, 270]
    rt = np.ascontiguousarray(
        rtcat.reshape(KC, P, 3 * DM).transpose(1, 0, 2)
    )  # [P, KC, 270]
    rt8 = np.ascontiguousarray(
        RS.astype(E4).T.reshape(KC, P, DM).transpose(1, 0, 2)
    )  # [P, KC, 90]

    f = np.asarray(feature, dtype=np.float32)
    fn = f / np.sqrt((f.astype(np.float64) ** 2).sum(axis=1, keepdims=True)).astype(
        np.float32
    )

    per_core = []
    for c in range(NCORES):
        fc = fn[c * BC : (c + 1) * BC]  # [4096, 768]
        fhi = fc.astype(np.float16)
        flo = (fc - fhi.astype(np.float32)) * np.float32(S8)
        # [p, t, k, cc] = fc[t*128+cc, k*128+p]
        fhi_t = np.ascontiguousarray(
            fhi.reshape(NT, P, KC, P).transpose(3, 0, 2, 1)
        )
        flo8_t = np.ascontiguousarray(
            flo.astype(E4).reshape(NT, P, KC, P).transpose(3, 0, 2, 1)
        )
        per_core.append({"fhi": fhi_t, "flo8": flo8_t, "rt": rt, "rt8": rt8})
    return per_core


def kernel(feature, category, W_topic, W_domain, memory):
    from concourse.bass_utils import run_bass_kernel_spmd

    in_maps = _host_prep(
        feature, np.asarray(W_topic), np.asarray(W_domain), np.asarray(memory)
    )
    nc = _get_nc()
    res = run_bass_kernel_spmd(nc, in_maps, core_ids=list(range(NCORES)))
    outs = [
        res.results[c]["out"].transpose(1, 0, 2).reshape(BC, D)
        for c in range(NCORES)
    ]
    full = np.concatenate(outs, axis=0)  # [B, 9]
    return full[:, None, :].astype(np.float32)
